# revision 1
# baseline (speedup 1.0000x reference)
"""Trainium2 Bass kernel for BinaryLinear: out = x @ sign(W).T + bias.

Full shapes: x (8192, 4096) f32, weight (4096, 4096) f32, bias (4096,) f32,
out (8192, 4096) f32.

Strategy: data-parallel shard of x over the 8192-token dim across 8 cores
(1024 tokens/core). Each core computes its token slice against the full
weight matrix, with a mixed-precision split of the 4096-deep contraction:
  - in-features [0, 2304): x and sign(W) as fp8 e4m3, contracted with
    perf_mode=DoubleRow (2 fp8 MACs/cell/cycle) — 9 paired matmuls of
    K=256 instead of 18 of K=128
  - in-features [2304, 4096): bf16 path (14 matmuls of K=128)
  This cuts PE work per PSUM group from 32 to 23 matmul-equivalents while
  keeping rel err ~1.987e-2 (< 2e-2 gate; fp8-only would be 2.67e-2;
  the HW error matches an ml_dtypes host simulation to ~2e-6, so the
  margin is deterministic, not statistical).
  x is pre-scaled by 2 (exact) and sign(W) is encoded as {+0.5, -0.5}
  (exact in both e4m3 and bf16), so products are exactly x*sign(w).
  - PE accumulates all 24 partial matmuls in f32 PSUM. PSUM is oriented
    [out_features, tokens] so bias is per-partition and the whole PSUM
    eviction (copy + bias add) is ONE exact ACT op; the core returns
    out.T and the host transposes back
  - first two output n-tiles run k-outer (8 interleaved PSUM groups) so PE
    streams while the x+W preload is still in flight; later n-tiles run
    group-outer with the next W panel prefetched during the previous tile

Engine assignment: PE matmul; DVE w-sign encode (fp8/bf16); ACT x-cast +
eviction; sync issues W DMAs (+ steady-state output DMAs); gpsimd issues
x DMAs + burst output DMAs.
"""

import sys

for _p in ("/opt/trn_rl_repo",):
    if _p not in sys.path:
        sys.path.append(_p)

import numpy as np

import concourse.mybir as mybir
import concourse.tile as tile
from concourse import bacc
from concourse.bass_utils import run_bass_kernel_spmd

P = 128
N_CORES = 8
T_FULL = 8192
D_IN = 4096
D_OUT = 4096
T_SHARD = T_FULL // N_CORES  # 1024
K_CH = D_IN // P  # 32 contraction chunks of 128
N_FP8_CH = 18  # chunks [0, 18) in fp8 (9 DoubleRow pairs)
N_DR = N_FP8_CH // 2  # 8
N_BF_CH = K_CH - N_FP8_CH  # 16 chunks in bf16
N_TILE = 512
N_TILES = D_OUT // N_TILE  # 8 output-feature tiles
O_SUB = N_TILE // P  # 4 psum groups along out_features per n-tile
T_HALF = 2  # 2 psum groups along tokens (512 each)
N_GROUPS = O_SUB * T_HALF  # 8 concurrent PSUM groups = all 8 banks

_compiled = None


def _build():
    nc = bacc.Bacc("TRN2", target_bir_lowering=False)
    f32 = mybir.dt.float32
    bf16 = mybir.dt.bfloat16
    fp8 = mybir.dt.float8e4
    DR = mybir.MatmulPerfMode.DoubleRow

    xT = nc.dram_tensor("xT", (D_IN, T_SHARD), f32, kind="ExternalInput")
    wT = nc.dram_tensor("wT", (D_IN, D_OUT), f32, kind="ExternalInput")
    # bias striped [128, 32]: column j holds bias[j*128 : (j+1)*128]
    bias_in = nc.dram_tensor("bias_col", (P, D_OUT // P), f32, kind="ExternalInput")
    # transposed output; host transposes back
    outT = nc.dram_tensor("outT", (D_OUT, T_SHARD), f32, kind="ExternalOutput")

    with tile.TileContext(nc) as tc:
        with (
            tc.tile_pool(name="const", bufs=1) as const,
            tc.tile_pool(name="xres", bufs=1) as xres,
            tc.tile_pool(name="xstg", bufs=4) as xstg,
            tc.tile_pool(name="w8res", bufs=2) as w8res,
            tc.tile_pool(name="wbres", bufs=2) as wbres,
            tc.tile_pool(name="wstg", bufs=8) as wstg,
            tc.tile_pool(name="opool", bufs=3) as opool,
            tc.tile_pool(name="psum", bufs=1, space="PSUM") as psum,
        ):
            bias_sb = const.tile([P, D_OUT // P], f32)
            nc.gpsimd.dma_start(bias_sb[:], bias_in[:])

            # PE warmup: throwaway matmuls while the first data chunks are in
            # flight, so real matmuls start at 2.4GHz (HAM warm)
            warm_l = const.tile([P, P], bf16)
            nc.vector.memset(warm_l[:], 1.0)
            warm_r = const.tile([P, N_TILE], bf16)
            nc.vector.memset(warm_r[:], 1.0)
            # all-zero moving operand: accumulating it into a live PSUM group
            # adds exactly 0.0 — used as a HAM keep-warm filler during the
            # DMA-bound x+W preload (else PE idles >50%/window and HAM
            # throttles the clock to 1.2GHz, making preload matmuls 2x slow)
            warm_z = const.tile([P, N_TILE], bf16)
            nc.vector.memset(warm_z[:], 0.0)
            # 12 x ~427ns cold = ~5.1us sustained busy: crosses the ~3.4us HAM
            # window so the clock is at 2.4GHz before the first real matmul
            ps_warm = psum.tile([P, N_TILE], f32, name="ps0", tag="ps0")
            for _ in range(12):
                nc.tensor.matmul(
                    ps_warm[:], warm_l[:], warm_r[:], start=True, stop=True
                )

            # resident x: fp8 pairs for chunks [0,16), bf16 for [16,32)
            x8 = xres.tile([P, N_DR, 2, T_SHARD], fp8)
            xbf = xres.tile([P, N_BF_CH, T_SHARD], bf16)

            def cast_x_chunk(k, xs):
                # 2x * 0.5sign == x * sign; factor 2 exact in fp8/bf16
                if k < N_FP8_CH:
                    dst = x8[:, k // 2, k % 2, :]
                else:
                    dst = xbf[:, k - N_FP8_CH, :]
                nc.scalar.activation(
                    dst, xs[:], mybir.ActivationFunctionType.Copy,
                    bias=0.0, scale=2.0,
                )

            def load_w_chunk(nt, k):
                ws = wstg.tile([P, N_TILE], f32, tag="ws")
                nc.sync.dma_start(
                    ws[:], wT[k * P : (k + 1) * P, nt * N_TILE : (nt + 1) * N_TILE]
                )
                # {+0.5, -0.5} = (w >= 0) - 0.5
                if k < N_FP8_CH:
                    dst = w8[:, k // 2, k % 2, :]
                else:
                    dst = wbf[:, k - N_FP8_CH, :]
                nc.vector.tensor_scalar(
                    dst, ws[:], 0.0, 0.5,
                    mybir.AluOpType.is_ge, mybir.AluOpType.subtract,
                )

            def mm_dr(k8, ps_list):
                # DoubleRow: slot i of lhsT pairs slot i of rhs; K=256
                for g in range(N_GROUPS):
                    o_sub, th = divmod(g, T_HALF)
                    nc.tensor.matmul(
                        ps_list[g][:],
                        w8[:, k8, 0:2, o_sub * P : (o_sub + 1) * P],
                        x8[:, k8, 0:2, th * N_TILE : (th + 1) * N_TILE],
                        start=(k8 == 0),
                        stop=False,
                        perf_mode=DR,
                    )

            def mm_bf(kb, ps_list):
                for g in range(N_GROUPS):
                    o_sub, th = divmod(g, T_HALF)
                    nc.tensor.matmul(
                        ps_list[g][:],
                        wbf[:, kb, o_sub * P : (o_sub + 1) * P],
                        xbf[:, kb, th * N_TILE : (th + 1) * N_TILE],
                        start=False,
                        stop=(kb == N_BF_CH - 1),
                    )

            def mm_for_chunk(k, ps_list):
                # k-outer form: fp8 pair fires once both slots are loaded
                if k < N_FP8_CH:
                    if k % 2 == 1:
                        mm_dr((k - 1) // 2, ps_list)
                else:
                    mm_bf(k - N_FP8_CH, ps_list)

            def evict(nt, g, ps, dma_engine, burst=False):
                # ONE exact ACT op: outT_tile = Identity(psum + bias[o])
                # burst evictions get per-group buffers so PSUM frees are
                # never paced by the output-DMA drain
                o_sub, th = divmod(g, T_HALF)
                o_idx = nt * O_SUB + o_sub
                if burst:
                    ot = opool.tile([P, N_TILE], f32, tag=f"otb{g}", bufs=1)
                else:
                    ot = opool.tile([P, N_TILE], f32, tag="ot")
                nc.scalar.activation(
                    ot[:], ps[:], mybir.ActivationFunctionType.Identity,
                    bias=bias_sb[:, o_idx : o_idx + 1],
                )
                dma_engine.dma_start(
                    outT[o_idx * P : (o_idx + 1) * P,
                         th * N_TILE : (th + 1) * N_TILE],
                    ot[:],
                )

            def alloc_psums():
                return [
                    psum.tile([P, N_TILE], f32, name=f"ps{g}", tag=f"ps{g}")
                    for g in range(N_GROUPS)
                ]

            # ---- nt = 0: fused x preload + k-outer matmul streaming ----
            w8 = w8res.tile([P, N_DR, 2, N_TILE], fp8, tag="w8")
            wbf = wbres.tile([P, N_BF_CH, N_TILE], bf16, tag="wbf")
            ps_l = alloc_psums()
            for k in range(K_CH):
                xs = xstg.tile([P, T_SHARD], f32, tag="xs")
                nc.gpsimd.dma_start(xs[:], xT[k * P : (k + 1) * P, :])
                cast_x_chunk(k, xs)
                load_w_chunk(0, k)
                mm_for_chunk(k, ps_l)
                # zero-accumulate keep-warm MMs: fill the per-chunk DMA
                # deficit (~0.4us) so HAM sees a busy PE and stays at 2.4GHz;
                # they have no data deps, so they run only while real MMs
                # would stall on the (HBM-saturated) preload stream
                if 2 <= k <= K_CH - 2:
                    for j in range(2):
                        nc.tensor.matmul(
                            ps_l[(k + 4 * j) % N_GROUPS][:],
                            warm_l[:], warm_z[:],
                            start=False, stop=False,
                        )

            # ---- nt = 1: k-outer (W still streaming, x resident) ----
            ps_l0 = ps_l
            w8 = w8res.tile([P, N_DR, 2, N_TILE], fp8, tag="w8")
            wbf = wbres.tile([P, N_BF_CH, N_TILE], bf16, tag="wbf")
            load_w_chunk(1, 0)
            for g in range(N_GROUPS):
                evict(0, g, ps_l0[g], nc.gpsimd, burst=True)
            ps_l = alloc_psums()
            for k in range(K_CH):
                if k > 0:
                    load_w_chunk(1, k)
                mm_for_chunk(k, ps_l)

            # ---- nt >= 2: group-outer, W panel prefetched during nt-1 ----
            for nt in range(2, N_TILES):
                ps_prev = ps_l
                w8 = w8res.tile([P, N_DR, 2, N_TILE], fp8, tag="w8")
                wbf = wbres.tile([P, N_BF_CH, N_TILE], bf16, tag="wbf")
                for k in range(K_CH):
                    load_w_chunk(nt, k)
                if nt == 2:
                    for g in range(N_GROUPS):
                        evict(1, g, ps_prev[g], nc.gpsimd, burst=True)
                ps_l = []
                for g in range(N_GROUPS):
                    o_sub, th = divmod(g, T_HALF)
                    ps = psum.tile([P, N_TILE], f32, name=f"ps{g}", tag=f"ps{g}")
                    for k8 in range(N_DR):
                        nc.tensor.matmul(
                            ps[:],
                            w8[:, k8, 0:2, o_sub * P : (o_sub + 1) * P],
                            x8[:, k8, 0:2, th * N_TILE : (th + 1) * N_TILE],
                            start=(k8 == 0),
                            stop=False,
                            perf_mode=DR,
                        )
                    for kb in range(N_BF_CH):
                        nc.tensor.matmul(
                            ps[:],
                            wbf[:, kb, o_sub * P : (o_sub + 1) * P],
                            xbf[:, kb, th * N_TILE : (th + 1) * N_TILE],
                            start=False,
                            stop=(kb == N_BF_CH - 1),
                        )
                    evict(nt, g, ps, nc.sync)

    nc.compile()
    return nc


def make_in_maps(x, weight, bias):
    x = np.asarray(x, dtype=np.float32)
    weight = np.asarray(weight, dtype=np.float32)
    bias = np.asarray(bias, dtype=np.float32)

    wT = np.ascontiguousarray(weight.T)
    bias_col = np.ascontiguousarray(bias.reshape(D_OUT // P, P).T)
    in_maps = []
    for c in range(N_CORES):
        xTc = np.ascontiguousarray(x[c * T_SHARD : (c + 1) * T_SHARD, :].T)
        in_maps.append({"xT": xTc, "wT": wT, "bias_col": bias_col})
    return in_maps


def _spot_check(out, x, weight, bias):
    # transient device glitches were observed (~1/10 runs returns garbage);
    # verify a few rows against the host and signal a retry if corrupted
    rows = [1, T_FULL // 3, (2 * T_FULL) // 3, T_FULL - 2]
    ref = x[rows].astype(np.float64) @ np.sign(weight).astype(np.float64).T + bias
    err = np.linalg.norm(out[rows].astype(np.float64) - ref) / np.linalg.norm(ref)
    return err < 5e-2


def kernel(x, weight, bias):
    global _compiled
    if _compiled is None:
        _compiled = _build()
    nc = _compiled

    in_maps = make_in_maps(x, weight, bias)
    for _attempt in range(3):
        res = run_bass_kernel_spmd(nc, in_maps, core_ids=list(range(N_CORES)))
        out = np.concatenate(
            [np.ascontiguousarray(res.results[c]["outT"].T) for c in range(N_CORES)],
            axis=0,
        )
        if _spot_check(out, x, weight, bias):
            break
    return out



# revision 3
# speedup vs baseline: 1.0512x; 1.0512x over previous
"""Trainium2 Bass kernel for BinaryLinear: out = x @ sign(W).T + bias.

Full shapes: x (8192, 4096) f32, weight (4096, 4096) f32, bias (4096,) f32,
out (8192, 4096) f32.

Strategy: data-parallel shard of x over the 8192-token dim across 8 cores
(1024 tokens/core). Each core computes its token slice against the full
weight matrix, with a mixed-precision split of the 4096-deep contraction:
  - in-features [0, 2304): x and sign(W) as fp8 e4m3, contracted with
    perf_mode=DoubleRow (2 fp8 MACs/cell/cycle) — 9 paired matmuls of
    K=256 instead of 18 of K=128
  - in-features [2304, 4096): bf16 path (14 matmuls of K=128)
  This cuts PE work per PSUM group from 32 to 23 matmul-equivalents while
  keeping rel err ~1.987e-2 (< 2e-2 gate; fp8-only would be 2.67e-2;
  the HW error matches an ml_dtypes host simulation to ~2e-6, so the
  margin is deterministic, not statistical).
  x is pre-scaled by 2 (exact) and sign(W) is encoded as {+0.5, -0.5}
  (exact in both e4m3 and bf16), so products are exactly x*sign(w).
  - PE accumulates all 24 partial matmuls in f32 PSUM. PSUM is oriented
    [out_features, tokens] so bias is per-partition and the whole PSUM
    eviction (copy + bias add) is ONE exact ACT op; the core returns
    out.T and the host transposes back
  - first two output n-tiles run k-outer (8 interleaved PSUM groups) so PE
    streams while the x+W preload is still in flight; later n-tiles run
    group-outer with the next W panel prefetched during the previous tile

Engine assignment: PE matmul; DVE w-sign encode (fp8/bf16); ACT x-cast +
eviction; sync issues W DMAs (+ steady-state output DMAs); gpsimd issues
x DMAs + burst output DMAs.
"""

import sys

for _p in ("/opt/trn_rl_repo",):
    if _p not in sys.path:
        sys.path.append(_p)

import numpy as np
import ml_dtypes

import concourse.mybir as mybir
import concourse.tile as tile
from concourse import bacc
from concourse.bass_utils import run_bass_kernel_spmd

P = 128
N_CORES = 8
T_FULL = 8192
D_IN = 4096
D_OUT = 4096
T_SHARD = T_FULL // N_CORES  # 1024
K_CH = D_IN // P  # 32 contraction chunks of 128
N_FP8_CH = 18  # chunks [0, 18) in fp8 (9 DoubleRow pairs)
N_DR = N_FP8_CH // 2  # 8
N_BF_CH = K_CH - N_FP8_CH  # 16 chunks in bf16
N_TILE = 512
N_TILES = D_OUT // N_TILE  # 8 output-feature tiles
O_SUB = N_TILE // P  # 4 psum groups along out_features per n-tile
T_HALF = 2  # 2 psum groups along tokens (512 each)
N_GROUPS = O_SUB * T_HALF  # 8 concurrent PSUM groups = all 8 banks

_compiled = None


def _build():
    nc = bacc.Bacc("TRN2", target_bir_lowering=False)
    f32 = mybir.dt.float32
    bf16 = mybir.dt.bfloat16
    fp8 = mybir.dt.float8e4
    DR = mybir.MatmulPerfMode.DoubleRow

    xT = nc.dram_tensor("xT", (D_IN, T_SHARD), f32, kind="ExternalInput")
    wT = nc.dram_tensor("wT", (D_IN, D_OUT), bf16, kind="ExternalInput")
    # bias striped [128, 32]: column j holds bias[j*128 : (j+1)*128]
    bias_in = nc.dram_tensor("bias_col", (P, D_OUT // P), f32, kind="ExternalInput")
    # transposed output; host transposes back
    outT = nc.dram_tensor("outT", (D_OUT, T_SHARD), f32, kind="ExternalOutput")

    with tile.TileContext(nc) as tc:
        with (
            tc.tile_pool(name="const", bufs=1) as const,
            tc.tile_pool(name="xres", bufs=1) as xres,
            tc.tile_pool(name="xstg", bufs=4) as xstg,
            tc.tile_pool(name="w8res", bufs=2) as w8res,
            tc.tile_pool(name="wbres", bufs=2) as wbres,
            tc.tile_pool(name="wstg", bufs=8) as wstg,
            tc.tile_pool(name="opool", bufs=3) as opool,
            tc.tile_pool(name="psum", bufs=1, space="PSUM") as psum,
        ):
            bias_sb = const.tile([P, D_OUT // P], f32)
            nc.gpsimd.dma_start(bias_sb[:], bias_in[:])

            # PE warmup: throwaway matmuls while the first data chunks are in
            # flight, so real matmuls start at 2.4GHz (HAM warm)
            warm_l = const.tile([P, P], bf16)
            nc.vector.memset(warm_l[:], 1.0)
            warm_r = const.tile([P, N_TILE], bf16)
            nc.vector.memset(warm_r[:], 1.0)
            # all-zero moving operand: accumulating it into a live PSUM group
            # adds exactly 0.0 — used as a HAM keep-warm filler during the
            # DMA-bound x+W preload (else PE idles >50%/window and HAM
            # throttles the clock to 1.2GHz, making preload matmuls 2x slow)
            warm_z = const.tile([P, N_TILE], bf16)
            nc.vector.memset(warm_z[:], 0.0)
            # 12 x ~427ns cold = ~5.1us sustained busy: crosses the ~3.4us HAM
            # window so the clock is at 2.4GHz before the first real matmul
            ps_warm = psum.tile([P, N_TILE], f32, name="ps0", tag="ps0")
            for _ in range(12):
                nc.tensor.matmul(
                    ps_warm[:], warm_l[:], warm_r[:], start=True, stop=True
                )

            # resident x: fp8 pairs for chunks [0,16), bf16 for [16,32)
            x8 = xres.tile([P, N_DR, 2, T_SHARD], fp8)
            xbf = xres.tile([P, N_BF_CH, T_SHARD], bf16)

            def cast_x_chunk(k, xs):
                # 2x * 0.5sign == x * sign; factor 2 exact in fp8/bf16
                if k < N_FP8_CH:
                    dst = x8[:, k // 2, k % 2, :]
                else:
                    dst = xbf[:, k - N_FP8_CH, :]
                nc.scalar.activation(
                    dst, xs[:], mybir.ActivationFunctionType.Copy,
                    bias=0.0, scale=2.0,
                )

            def load_w_chunk(nt, k):
                # {+0.5, -0.5} = (w >= 0) - 0.5; fp8 dst is o_sub-major so
                # each DR lhsT [K, 2, 128] is a contiguous slot-stride-128
                # slice (contiguous LDWEIGHTS runs ~216ns/DR vs ~241ns for
                # the strided form)
                if k < N_FP8_CH:
                    ws = wstg.tile([P, O_SUB, P], bf16, tag="ws4")
                    nc.sync.dma_start(
                        ws[:],
                        wT[k * P : (k + 1) * P,
                           nt * N_TILE : (nt + 1) * N_TILE],
                    )
                    dst = w8[:, k // 2, :, k % 2, :]
                else:
                    ws = wstg.tile([P, N_TILE], bf16, tag="ws2")
                    nc.sync.dma_start(
                        ws[:],
                        wT[k * P : (k + 1) * P,
                           nt * N_TILE : (nt + 1) * N_TILE],
                    )
                    dst = wbf[:, k - N_FP8_CH, :]
                nc.vector.tensor_scalar(
                    dst, ws[:], 0.0, 0.5,
                    mybir.AluOpType.is_ge, mybir.AluOpType.subtract,
                )

            def mm_dr(k8, ps_list):
                # DoubleRow: slot i of lhsT pairs slot i of rhs; K=256
                for g in range(N_GROUPS):
                    o_sub, th = divmod(g, T_HALF)
                    nc.tensor.matmul(
                        ps_list[g][:],
                        w8[:, k8, o_sub, 0:2, :],
                        x8[:, k8, 0:2, th * N_TILE : (th + 1) * N_TILE],
                        start=(k8 == 0),
                        stop=False,
                        perf_mode=DR,
                    )

            def mm_bf(kb, ps_list):
                for g in range(N_GROUPS):
                    o_sub, th = divmod(g, T_HALF)
                    nc.tensor.matmul(
                        ps_list[g][:],
                        wbf[:, kb, o_sub * P : (o_sub + 1) * P],
                        xbf[:, kb, th * N_TILE : (th + 1) * N_TILE],
                        start=False,
                        stop=(kb == N_BF_CH - 1),
                    )

            def mm_for_chunk(k, ps_list):
                # k-outer form: fp8 pair fires once both slots are loaded
                if k < N_FP8_CH:
                    if k % 2 == 1:
                        mm_dr((k - 1) // 2, ps_list)
                else:
                    mm_bf(k - N_FP8_CH, ps_list)

            def evict(nt, g, ps, dma_engine, burst=False):
                # ONE exact ACT op: outT_tile = Identity(psum + bias[o])
                # burst evictions get per-group buffers so PSUM frees are
                # never paced by the output-DMA drain
                o_sub, th = divmod(g, T_HALF)
                o_idx = nt * O_SUB + o_sub
                if burst:
                    ot = opool.tile([P, N_TILE], f32, tag=f"otb{g}", bufs=1)
                else:
                    ot = opool.tile([P, N_TILE], f32, tag="ot")
                nc.scalar.activation(
                    ot[:], ps[:], mybir.ActivationFunctionType.Identity,
                    bias=bias_sb[:, o_idx : o_idx + 1],
                )
                dma_engine.dma_start(
                    outT[o_idx * P : (o_idx + 1) * P,
                         th * N_TILE : (th + 1) * N_TILE],
                    ot[:],
                )

            def alloc_psums():
                return [
                    psum.tile([P, N_TILE], f32, name=f"ps{g}", tag=f"ps{g}")
                    for g in range(N_GROUPS)
                ]

            # ---- nt = 0: fused x preload + k-outer matmul streaming ----
            w8 = w8res.tile([P, N_DR, O_SUB, 2, P], fp8, tag="w8")
            wbf = wbres.tile([P, N_BF_CH, N_TILE], bf16, tag="wbf")
            ps_l = alloc_psums()
            for k in range(K_CH):
                xs = xstg.tile([P, T_SHARD], f32, tag="xs")
                nc.gpsimd.dma_start(xs[:], xT[k * P : (k + 1) * P, :])
                cast_x_chunk(k, xs)
                load_w_chunk(0, k)
                mm_for_chunk(k, ps_l)
                # zero-accumulate keep-warm MMs: fill the per-chunk DMA
                # deficit (~0.4us) so HAM sees a busy PE and stays at 2.4GHz;
                # they have no data deps, so they run only while real MMs
                # would stall on the (HBM-saturated) preload stream
                if 2 <= k <= K_CH - 2:
                    for j in range(2):
                        nc.tensor.matmul(
                            ps_l[(k + 4 * j) % N_GROUPS][:],
                            warm_l[:], warm_z[:],
                            start=False, stop=False,
                        )

            # ---- nt = 1: k-outer (W still streaming, x resident) ----
            ps_l0 = ps_l
            w8 = w8res.tile([P, N_DR, O_SUB, 2, P], fp8, tag="w8")
            wbf = wbres.tile([P, N_BF_CH, N_TILE], bf16, tag="wbf")
            load_w_chunk(1, 0)
            for g in range(N_GROUPS):
                evict(0, g, ps_l0[g], nc.gpsimd, burst=True)
            ps_l = alloc_psums()
            for k in range(K_CH):
                if k > 0:
                    load_w_chunk(1, k)
                mm_for_chunk(k, ps_l)

            # ---- nt >= 2: group-outer, W panel prefetched during nt-1 ----
            for nt in range(2, N_TILES):
                ps_prev = ps_l
                w8 = w8res.tile([P, N_DR, O_SUB, 2, P], fp8, tag="w8")
                wbf = wbres.tile([P, N_BF_CH, N_TILE], bf16, tag="wbf")
                for k in range(K_CH):
                    load_w_chunk(nt, k)
                if nt == 2:
                    for g in range(N_GROUPS):
                        evict(1, g, ps_prev[g], nc.gpsimd, burst=True)
                ps_l = []
                for g in range(N_GROUPS):
                    o_sub, th = divmod(g, T_HALF)
                    ps = psum.tile([P, N_TILE], f32, name=f"ps{g}", tag=f"ps{g}")
                    for k8 in range(N_DR):
                        nc.tensor.matmul(
                            ps[:],
                            w8[:, k8, o_sub, 0:2, :],
                            x8[:, k8, 0:2, th * N_TILE : (th + 1) * N_TILE],
                            start=(k8 == 0),
                            stop=False,
                            perf_mode=DR,
                        )
                    for kb in range(N_BF_CH):
                        nc.tensor.matmul(
                            ps[:],
                            wbf[:, kb, o_sub * P : (o_sub + 1) * P],
                            xbf[:, kb, th * N_TILE : (th + 1) * N_TILE],
                            start=False,
                            stop=(kb == N_BF_CH - 1),
                        )
                    evict(nt, g, ps, nc.sync)

    nc.compile()
    return nc


def make_in_maps(x, weight, bias):
    x = np.asarray(x, dtype=np.float32)
    weight = np.asarray(weight, dtype=np.float32)
    bias = np.asarray(bias, dtype=np.float32)

    wT = np.ascontiguousarray(weight.T).astype(ml_dtypes.bfloat16)
    bias_col = np.ascontiguousarray(bias.reshape(D_OUT // P, P).T)
    in_maps = []
    for c in range(N_CORES):
        xTc = np.ascontiguousarray(x[c * T_SHARD : (c + 1) * T_SHARD, :].T)
        in_maps.append({"xT": xTc, "wT": wT, "bias_col": bias_col})
    return in_maps


def _spot_check(out, x, weight, bias):
    # transient device glitches were observed (~1/10 runs returns garbage);
    # verify a few rows against the host and signal a retry if corrupted
    rows = [1, T_FULL // 3, (2 * T_FULL) // 3, T_FULL - 2]
    ref = x[rows].astype(np.float64) @ np.sign(weight).astype(np.float64).T + bias
    err = np.linalg.norm(out[rows].astype(np.float64) - ref) / np.linalg.norm(ref)
    return err < 5e-2


def kernel(x, weight, bias):
    global _compiled
    if _compiled is None:
        _compiled = _build()
    nc = _compiled

    in_maps = make_in_maps(x, weight, bias)
    for _attempt in range(3):
        res = run_bass_kernel_spmd(nc, in_maps, core_ids=list(range(N_CORES)))
        out = np.concatenate(
            [np.ascontiguousarray(res.results[c]["outT"].T) for c in range(N_CORES)],
            axis=0,
        )
        if _spot_check(out, x, weight, bias):
            break
    return out



# revision 4
# speedup vs baseline: 1.1308x; 1.0758x over previous
"""Trainium2 Bass kernel for BinaryLinear: out = x @ sign(W).T + bias.

Full shapes: x (8192, 4096) f32, weight (4096, 4096) f32, bias (4096,) f32,
out (8192, 4096) f32.

Strategy: data-parallel shard of x over the 8192-token dim across 8 cores
(1024 tokens/core). Each core computes its token slice against the full
weight matrix, with a mixed-precision split of the 4096-deep contraction:
  - in-features [0, 2304): x and sign(W) as fp8 e4m3, contracted with
    perf_mode=DoubleRow (2 fp8 MACs/cell/cycle) — 9 paired matmuls of
    K=256 instead of 18 of K=128
  - in-features [2304, 4096): bf16 path (14 matmuls of K=128)
  This cuts PE work per PSUM group from 32 to 23 matmul-equivalents while
  keeping rel err ~1.987e-2 (< 2e-2 gate; fp8-only would be 2.67e-2;
  the HW error matches an ml_dtypes host simulation to ~2e-6, so the
  margin is deterministic, not statistical).
  x is pre-scaled by 2 (exact) and sign(W) is encoded as {+0.5, -0.5}
  (exact in both e4m3 and bf16), so products are exactly x*sign(w).
  - PE accumulates all 24 partial matmuls in f32 PSUM. PSUM is oriented
    [out_features, tokens] so bias is per-partition and the whole PSUM
    eviction (copy + bias add) is ONE exact ACT op; the core returns
    out.T and the host transposes back
  - first two output n-tiles run k-outer (8 interleaved PSUM groups) so PE
    streams while the x+W preload is still in flight; later n-tiles run
    group-outer with the next W panel prefetched during the previous tile

Engine assignment: PE matmul; DVE w-sign encode (fp8/bf16); ACT x-cast +
eviction; sync issues W DMAs (+ steady-state output DMAs); gpsimd issues
x DMAs + burst output DMAs.
"""

import sys

for _p in ("/opt/trn_rl_repo",):
    if _p not in sys.path:
        sys.path.append(_p)

import numpy as np
import ml_dtypes

import concourse.mybir as mybir
import concourse.tile as tile
from concourse import bacc
from concourse.bass_utils import run_bass_kernel_spmd

P = 128
N_CORES = 8
T_FULL = 8192
D_IN = 4096
D_OUT = 4096
T_SHARD = T_FULL // N_CORES  # 1024
K_CH = D_IN // P  # 32 contraction chunks of 128
N_FP8_CH = 18  # chunks [0, 18) in fp8 (9 DoubleRow pairs)
N_DR = N_FP8_CH // 2  # 8
N_BF_CH = K_CH - N_FP8_CH  # 16 chunks in bf16
N_TILE = 512
N_TILES = D_OUT // N_TILE  # 8 output-feature tiles
O_SUB = N_TILE // P  # 4 psum groups along out_features per n-tile
T_HALF = 2  # 2 psum groups along tokens (512 each)
N_GROUPS = O_SUB * T_HALF  # 8 concurrent PSUM groups = all 8 banks

_compiled = None


def _build():
    nc = bacc.Bacc("TRN2", target_bir_lowering=False)
    f32 = mybir.dt.float32
    bf16 = mybir.dt.bfloat16
    fp8 = mybir.dt.float8e4
    DR = mybir.MatmulPerfMode.DoubleRow

    x8in = nc.dram_tensor(
        "x8in", (P, N_DR, 2, T_SHARD), fp8, kind="ExternalInput"
    )
    xbfin = nc.dram_tensor(
        "xbfin", (P, N_BF_CH, T_SHARD), bf16, kind="ExternalInput"
    )
    wT = nc.dram_tensor("wT", (D_IN, D_OUT), bf16, kind="ExternalInput")
    # bias striped [128, 32]: column j holds bias[j*128 : (j+1)*128]
    bias_in = nc.dram_tensor("bias_col", (P, D_OUT // P), f32, kind="ExternalInput")
    # transposed output; host transposes back
    outT = nc.dram_tensor("outT", (D_OUT, T_SHARD), f32, kind="ExternalOutput")

    with tile.TileContext(nc) as tc:
        with (
            tc.tile_pool(name="const", bufs=1) as const,
            tc.tile_pool(name="xres", bufs=1) as xres,
            tc.tile_pool(name="w8res", bufs=2) as w8res,
            tc.tile_pool(name="wbres", bufs=2) as wbres,
            tc.tile_pool(name="wstg", bufs=8) as wstg,
            tc.tile_pool(name="opool", bufs=3) as opool,
            tc.tile_pool(name="psum", bufs=1, space="PSUM") as psum,
        ):
            bias_sb = const.tile([P, D_OUT // P], f32)
            nc.gpsimd.dma_start(bias_sb[:], bias_in[:])

            # PE warmup: throwaway matmuls while the first data chunks are in
            # flight, so real matmuls start at 2.4GHz (HAM warm)
            warm_l = const.tile([P, P], bf16)
            nc.vector.memset(warm_l[:], 1.0)
            warm_r = const.tile([P, N_TILE], bf16)
            nc.vector.memset(warm_r[:], 1.0)
            # 12 x ~427ns cold = ~5.1us sustained busy: crosses the ~3.4us HAM
            # window so the clock is at 2.4GHz before the first real matmul
            ps_warm = psum.tile([P, N_TILE], f32, name="ps0", tag="ps0")
            for _ in range(12):
                nc.tensor.matmul(
                    ps_warm[:], warm_l[:], warm_r[:], start=True, stop=True
                )

            # resident x, pre-cast on host: fp8(2x) pairs + bf16(2x);
            # DMA straight in (5.75 MiB vs 16 MiB f32 -> no prologue DMA
            # wall, no ACT cast, no HAM keep-warm fillers needed)
            x8 = xres.tile([P, N_DR, 2, T_SHARD], fp8)
            xbf = xres.tile([P, N_BF_CH, T_SHARD], bf16)
            for k8 in range(N_DR):
                nc.gpsimd.dma_start(x8[:, k8, :, :], x8in[:, k8, :, :])
            for kb in range(N_BF_CH):
                nc.gpsimd.dma_start(xbf[:, kb, :], xbfin[:, kb, :])

            def load_w_chunk(nt, k):
                # {+0.5, -0.5} = (w >= 0) - 0.5; fp8 dst is o_sub-major so
                # each DR lhsT [K, 2, 128] is a contiguous slot-stride-128
                # slice (contiguous LDWEIGHTS runs ~216ns/DR vs ~241ns for
                # the strided form)
                if k < N_FP8_CH:
                    ws = wstg.tile([P, O_SUB, P], bf16, tag="ws4")
                    nc.sync.dma_start(
                        ws[:],
                        wT[k * P : (k + 1) * P,
                           nt * N_TILE : (nt + 1) * N_TILE],
                    )
                    dst = w8[:, k // 2, :, k % 2, :]
                else:
                    ws = wstg.tile([P, N_TILE], bf16, tag="ws2")
                    nc.sync.dma_start(
                        ws[:],
                        wT[k * P : (k + 1) * P,
                           nt * N_TILE : (nt + 1) * N_TILE],
                    )
                    dst = wbf[:, k - N_FP8_CH, :]
                nc.vector.tensor_scalar(
                    dst, ws[:], 0.0, 0.5,
                    mybir.AluOpType.is_ge, mybir.AluOpType.subtract,
                )

            def mm_dr(k8, ps_list):
                # DoubleRow: slot i of lhsT pairs slot i of rhs; K=256
                for g in range(N_GROUPS):
                    o_sub, th = divmod(g, T_HALF)
                    nc.tensor.matmul(
                        ps_list[g][:],
                        w8[:, k8, o_sub, 0:2, :],
                        x8[:, k8, 0:2, th * N_TILE : (th + 1) * N_TILE],
                        start=(k8 == 0),
                        stop=False,
                        perf_mode=DR,
                    )

            def mm_bf(kb, ps_list):
                for g in range(N_GROUPS):
                    o_sub, th = divmod(g, T_HALF)
                    nc.tensor.matmul(
                        ps_list[g][:],
                        wbf[:, kb, o_sub * P : (o_sub + 1) * P],
                        xbf[:, kb, th * N_TILE : (th + 1) * N_TILE],
                        start=False,
                        stop=(kb == N_BF_CH - 1),
                    )

            def mm_for_chunk(k, ps_list):
                # k-outer form: fp8 pair fires once both slots are loaded
                if k < N_FP8_CH:
                    if k % 2 == 1:
                        mm_dr((k - 1) // 2, ps_list)
                else:
                    mm_bf(k - N_FP8_CH, ps_list)

            def evict(nt, g, ps, dma_engine, burst=False):
                # ONE exact ACT op: outT_tile = Identity(psum + bias[o])
                # burst evictions get per-group buffers so PSUM frees are
                # never paced by the output-DMA drain
                o_sub, th = divmod(g, T_HALF)
                o_idx = nt * O_SUB + o_sub
                if burst:
                    ot = opool.tile([P, N_TILE], f32, tag=f"otb{g}", bufs=1)
                else:
                    ot = opool.tile([P, N_TILE], f32, tag="ot")
                nc.scalar.activation(
                    ot[:], ps[:], mybir.ActivationFunctionType.Identity,
                    bias=bias_sb[:, o_idx : o_idx + 1],
                )
                dma_engine.dma_start(
                    outT[o_idx * P : (o_idx + 1) * P,
                         th * N_TILE : (th + 1) * N_TILE],
                    ot[:],
                )

            def alloc_psums():
                return [
                    psum.tile([P, N_TILE], f32, name=f"ps{g}", tag=f"ps{g}")
                    for g in range(N_GROUPS)
                ]

            # ---- nt = 0: fused x preload + k-outer matmul streaming ----
            w8 = w8res.tile([P, N_DR, O_SUB, 2, P], fp8, tag="w8")
            wbf = wbres.tile([P, N_BF_CH, N_TILE], bf16, tag="wbf")
            ps_l = alloc_psums()
            for k in range(K_CH):
                load_w_chunk(0, k)
                mm_for_chunk(k, ps_l)

            # ---- nt = 1: k-outer (W still streaming, x resident) ----
            ps_l0 = ps_l
            w8 = w8res.tile([P, N_DR, O_SUB, 2, P], fp8, tag="w8")
            wbf = wbres.tile([P, N_BF_CH, N_TILE], bf16, tag="wbf")
            load_w_chunk(1, 0)
            for g in range(N_GROUPS):
                evict(0, g, ps_l0[g], nc.gpsimd, burst=True)
            ps_l = alloc_psums()
            for k in range(K_CH):
                if k > 0:
                    load_w_chunk(1, k)
                mm_for_chunk(k, ps_l)

            # ---- nt >= 2: group-outer, W panel prefetched during nt-1 ----
            for nt in range(2, N_TILES):
                ps_prev = ps_l
                w8 = w8res.tile([P, N_DR, O_SUB, 2, P], fp8, tag="w8")
                wbf = wbres.tile([P, N_BF_CH, N_TILE], bf16, tag="wbf")
                for k in range(K_CH):
                    load_w_chunk(nt, k)
                if nt == 2:
                    for g in range(N_GROUPS):
                        evict(1, g, ps_prev[g], nc.gpsimd, burst=True)
                ps_l = []
                for g in range(N_GROUPS):
                    o_sub, th = divmod(g, T_HALF)
                    ps = psum.tile([P, N_TILE], f32, name=f"ps{g}", tag=f"ps{g}")
                    for k8 in range(N_DR):
                        nc.tensor.matmul(
                            ps[:],
                            w8[:, k8, o_sub, 0:2, :],
                            x8[:, k8, 0:2, th * N_TILE : (th + 1) * N_TILE],
                            start=(k8 == 0),
                            stop=False,
                            perf_mode=DR,
                        )
                    for kb in range(N_BF_CH):
                        nc.tensor.matmul(
                            ps[:],
                            wbf[:, kb, o_sub * P : (o_sub + 1) * P],
                            xbf[:, kb, th * N_TILE : (th + 1) * N_TILE],
                            start=False,
                            stop=(kb == N_BF_CH - 1),
                        )
                    evict(nt, g, ps, nc.sync)

    nc.compile()
    return nc


_FP8_NP = np.dtype(mybir.dt.np(mybir.dt.float8e4))
_BF16_NP = np.dtype(mybir.dt.np(mybir.dt.bfloat16))


def make_in_maps(x, weight, bias):
    x = np.asarray(x, dtype=np.float32)
    weight = np.asarray(weight, dtype=np.float32)
    bias = np.asarray(bias, dtype=np.float32)

    wT = np.ascontiguousarray(weight.T).astype(_BF16_NP)
    bias_col = np.ascontiguousarray(bias.reshape(D_OUT // P, P).T)
    in_maps = []
    for c in range(N_CORES):
        x2 = 2.0 * x[c * T_SHARD : (c + 1) * T_SHARD, :].T  # [D_IN, T]
        x8c = np.ascontiguousarray(
            x2[: N_FP8_CH * P]
            .reshape(N_DR, 2, P, T_SHARD)
            .transpose(2, 0, 1, 3)
        ).astype(_FP8_NP)
        xbfc = np.ascontiguousarray(
            x2[N_FP8_CH * P :]
            .reshape(N_BF_CH, P, T_SHARD)
            .transpose(1, 0, 2)
        ).astype(_BF16_NP)
        in_maps.append(
            {"x8in": x8c, "xbfin": xbfc, "wT": wT, "bias_col": bias_col}
        )
    return in_maps


def _spot_check(out, x, weight, bias):
    # transient device glitches were observed (~1/10 runs returns garbage);
    # verify a few rows against the host and signal a retry if corrupted
    rows = [1, T_FULL // 3, (2 * T_FULL) // 3, T_FULL - 2]
    ref = x[rows].astype(np.float64) @ np.sign(weight).astype(np.float64).T + bias
    err = np.linalg.norm(out[rows].astype(np.float64) - ref) / np.linalg.norm(ref)
    return err < 5e-2


def kernel(x, weight, bias):
    global _compiled
    if _compiled is None:
        _compiled = _build()
    nc = _compiled

    in_maps = make_in_maps(x, weight, bias)
    for _attempt in range(3):
        res = run_bass_kernel_spmd(nc, in_maps, core_ids=list(range(N_CORES)))
        out = np.concatenate(
            [np.ascontiguousarray(res.results[c]["outT"].T) for c in range(N_CORES)],
            axis=0,
        )
        if _spot_check(out, x, weight, bias):
            break
    return out



# revision 5
# speedup vs baseline: 1.1415x; 1.0095x over previous
"""Trainium2 Bass kernel for BinaryLinear: out = x @ sign(W).T + bias.

Full shapes: x (8192, 4096) f32, weight (4096, 4096) f32, bias (4096,) f32,
out (8192, 4096) f32.

Strategy: data-parallel shard of x over the 8192-token dim across 8 cores
(1024 tokens/core). Each core computes its token slice against the full
weight matrix, with a mixed-precision split of the 4096-deep contraction:
  - in-features [0, 2304): x and sign(W) as fp8 e4m3, contracted with
    perf_mode=DoubleRow (2 fp8 MACs/cell/cycle) — 9 paired matmuls of
    K=256 instead of 18 of K=128
  - in-features [2304, 4096): bf16 path (14 matmuls of K=128)
  This cuts PE work per PSUM group from 32 to 23 matmul-equivalents while
  keeping rel err ~1.987e-2 (< 2e-2 gate; fp8-only would be 2.67e-2;
  the HW error matches an ml_dtypes host simulation to ~2e-6, so the
  margin is deterministic, not statistical).
  x is pre-scaled by 2 (exact) and sign(W) is encoded as {+0.5, -0.5}
  (exact in both e4m3 and bf16), so products are exactly x*sign(w).
  - PE accumulates all 24 partial matmuls in f32 PSUM. PSUM is oriented
    [out_features, tokens] so bias is per-partition and the whole PSUM
    eviction (copy + bias add) is ONE exact ACT op; the core returns
    out.T and the host transposes back
  - first two output n-tiles run k-outer (8 interleaved PSUM groups) so PE
    streams while the x+W preload is still in flight; later n-tiles run
    group-outer with the next W panel prefetched during the previous tile

Engine assignment: PE matmul; DVE w-sign encode (fp8/bf16); ACT x-cast +
eviction; sync issues W DMAs (+ steady-state output DMAs); gpsimd issues
x DMAs + burst output DMAs.
"""

import sys

for _p in ("/opt/trn_rl_repo",):
    if _p not in sys.path:
        sys.path.append(_p)

import numpy as np
import ml_dtypes

import concourse.mybir as mybir
import concourse.tile as tile
from concourse import bacc
from concourse.bass_utils import run_bass_kernel_spmd

P = 128
N_CORES = 8
T_FULL = 8192
D_IN = 4096
D_OUT = 4096
T_SHARD = T_FULL // N_CORES  # 1024
K_CH = D_IN // P  # 32 contraction chunks of 128
N_FP8_CH = 18  # chunks [0, 18) in fp8 (9 DoubleRow pairs)
N_DR = N_FP8_CH // 2  # 8
N_BF_CH = K_CH - N_FP8_CH  # 16 chunks in bf16
N_TILE = 512
N_TILES = D_OUT // N_TILE  # 8 output-feature tiles
O_SUB = N_TILE // P  # 4 psum groups along out_features per n-tile
T_HALF = 2  # 2 psum groups along tokens (512 each)
N_GROUPS = O_SUB * T_HALF  # 8 concurrent PSUM groups = all 8 banks

_compiled = None


def _build():
    nc = bacc.Bacc("TRN2", target_bir_lowering=False)
    f32 = mybir.dt.float32
    bf16 = mybir.dt.bfloat16
    fp8 = mybir.dt.float8e4
    DR = mybir.MatmulPerfMode.DoubleRow

    x8in = nc.dram_tensor(
        "x8in", (P, N_DR, 2, T_SHARD), fp8, kind="ExternalInput"
    )
    xbfin = nc.dram_tensor(
        "xbfin", (P, N_BF_CH, T_SHARD), bf16, kind="ExternalInput"
    )
    wT = nc.dram_tensor("wT", (D_IN, D_OUT), bf16, kind="ExternalInput")
    # bias striped [128, 32]: column j holds bias[j*128 : (j+1)*128]
    bias_in = nc.dram_tensor("bias_col", (P, D_OUT // P), f32, kind="ExternalInput")
    # transposed output; host transposes back
    outT = nc.dram_tensor("outT", (D_OUT, T_SHARD), f32, kind="ExternalOutput")

    with tile.TileContext(nc) as tc:
        with (
            tc.tile_pool(name="const", bufs=1) as const,
            tc.tile_pool(name="xres", bufs=1) as xres,
            tc.tile_pool(name="w8res", bufs=2) as w8res,
            tc.tile_pool(name="wbres", bufs=2) as wbres,
            tc.tile_pool(name="wstg", bufs=8) as wstg,
            tc.tile_pool(name="opool", bufs=3) as opool,
            tc.tile_pool(name="psum", bufs=1, space="PSUM") as psum,
        ):
            bias_sb = const.tile([P, D_OUT // P], f32)
            nc.gpsimd.dma_start(bias_sb[:], bias_in[:])

            # PE warmup: throwaway matmuls while the first data chunks are in
            # flight, so real matmuls start at 2.4GHz (HAM warm)
            warm_l = const.tile([P, P], bf16)
            nc.vector.memset(warm_l[:], 1.0)
            warm_r = const.tile([P, N_TILE], bf16)
            nc.vector.memset(warm_r[:], 1.0)
            # ~6 x ~427ns cold: ramp the clock while the first x/W
            # chunks are in flight (preload is no longer DMA-bound, so the
            # ramp finishes on early real matmuls)
            ps_warm = psum.tile([P, N_TILE], f32, name="ps0", tag="ps0")
            for _ in range(6):
                nc.tensor.matmul(
                    ps_warm[:], warm_l[:], warm_r[:], start=True, stop=True
                )

            # resident x, pre-cast on host: fp8(2x) pairs + bf16(2x);
            # DMA straight in (5.75 MiB vs 16 MiB f32 -> no prologue DMA
            # wall, no ACT cast, no HAM keep-warm fillers needed)
            x8 = xres.tile([P, N_DR, 2, T_SHARD], fp8)
            xbf = xres.tile([P, N_BF_CH, T_SHARD], bf16)
            _b = 0
            for k8 in range(N_DR):
                nc.gpsimd.dma_start(x8[:, k8, :, :], x8in[:, k8, :, :])
                if _b < N_BF_CH:
                    nc.gpsimd.dma_start(xbf[:, _b, :], xbfin[:, _b, :])
                    _b += 1
            while _b < N_BF_CH:
                nc.gpsimd.dma_start(xbf[:, _b, :], xbfin[:, _b, :])
                _b += 1

            def load_w_chunk(nt, k):
                # {+0.5, -0.5} = (w >= 0) - 0.5; fp8 dst is o_sub-major so
                # each DR lhsT [K, 2, 128] is a contiguous slot-stride-128
                # slice (contiguous LDWEIGHTS runs ~216ns/DR vs ~241ns for
                # the strided form)
                if k < N_FP8_CH:
                    ws = wstg.tile([P, O_SUB, P], bf16, tag="ws4")
                    nc.sync.dma_start(
                        ws[:],
                        wT[k * P : (k + 1) * P,
                           nt * N_TILE : (nt + 1) * N_TILE],
                    )
                    dst = w8[:, k // 2, :, k % 2, :]
                else:
                    ws = wstg.tile([P, N_TILE], bf16, tag="ws2")
                    nc.sync.dma_start(
                        ws[:],
                        wT[k * P : (k + 1) * P,
                           nt * N_TILE : (nt + 1) * N_TILE],
                    )
                    dst = wbf[:, k - N_FP8_CH, :]
                nc.vector.tensor_scalar(
                    dst, ws[:], 0.0, 0.5,
                    mybir.AluOpType.is_ge, mybir.AluOpType.subtract,
                )

            def mm_dr(k8, ps_list):
                # DoubleRow: slot i of lhsT pairs slot i of rhs; K=256
                for g in range(N_GROUPS):
                    o_sub, th = divmod(g, T_HALF)
                    nc.tensor.matmul(
                        ps_list[g][:],
                        w8[:, k8, o_sub, 0:2, :],
                        x8[:, k8, 0:2, th * N_TILE : (th + 1) * N_TILE],
                        start=(k8 == 0),
                        stop=False,
                        perf_mode=DR,
                    )

            def mm_bf(kb, ps_list):
                for g in range(N_GROUPS):
                    o_sub, th = divmod(g, T_HALF)
                    nc.tensor.matmul(
                        ps_list[g][:],
                        wbf[:, kb, o_sub * P : (o_sub + 1) * P],
                        xbf[:, kb, th * N_TILE : (th + 1) * N_TILE],
                        start=False,
                        stop=(kb == N_BF_CH - 1),
                    )

            def mm_for_chunk(k, ps_list):
                # k-outer form: fp8 pair fires once both slots are loaded
                if k < N_FP8_CH:
                    if k % 2 == 1:
                        mm_dr((k - 1) // 2, ps_list)
                else:
                    mm_bf(k - N_FP8_CH, ps_list)

            def evict(nt, g, ps, dma_engine, burst=False):
                # ONE exact ACT op: outT_tile = Identity(psum + bias[o])
                # burst evictions get per-group buffers so PSUM frees are
                # never paced by the output-DMA drain
                o_sub, th = divmod(g, T_HALF)
                o_idx = nt * O_SUB + o_sub
                if burst:
                    ot = opool.tile([P, N_TILE], f32, tag=f"otb{g}", bufs=1)
                else:
                    ot = opool.tile([P, N_TILE], f32, tag="ot")
                nc.scalar.activation(
                    ot[:], ps[:], mybir.ActivationFunctionType.Identity,
                    bias=bias_sb[:, o_idx : o_idx + 1],
                )
                dma_engine.dma_start(
                    outT[o_idx * P : (o_idx + 1) * P,
                         th * N_TILE : (th + 1) * N_TILE],
                    ot[:],
                )

            def alloc_psums():
                return [
                    psum.tile([P, N_TILE], f32, name=f"ps{g}", tag=f"ps{g}")
                    for g in range(N_GROUPS)
                ]

            # ---- nt = 0: fused x preload + k-outer matmul streaming ----
            w8 = w8res.tile([P, N_DR, O_SUB, 2, P], fp8, tag="w8")
            wbf = wbres.tile([P, N_BF_CH, N_TILE], bf16, tag="wbf")
            # interleave fp8 pairs with bf16 chunks so nt0's per-chunk
            # DMA demand (x+W) tracks per-chunk PE supply (fp8 chunks are
            # 2x cheaper on PE but equally DMA-heavy)
            sched0 = []
            _b = 0
            for k8 in range(N_DR):
                sched0 += [2 * k8, 2 * k8 + 1]
                if _b < N_BF_CH:
                    sched0.append(N_FP8_CH + _b)
                    _b += 1
            while _b < N_BF_CH:
                sched0.append(N_FP8_CH + _b)
                _b += 1
            ps_l = alloc_psums()
            for k in sched0:
                load_w_chunk(0, k)
                mm_for_chunk(k, ps_l)

            # ---- nt = 1: k-outer (W still streaming, x resident) ----
            ps_l0 = ps_l
            w8 = w8res.tile([P, N_DR, O_SUB, 2, P], fp8, tag="w8")
            wbf = wbres.tile([P, N_BF_CH, N_TILE], bf16, tag="wbf")
            load_w_chunk(1, 0)
            for g in range(N_GROUPS):
                evict(0, g, ps_l0[g], nc.gpsimd, burst=True)
            ps_l = alloc_psums()
            for k in range(K_CH):
                if k > 0:
                    load_w_chunk(1, k)
                mm_for_chunk(k, ps_l)

            # ---- nt >= 2: group-outer, W panel prefetched during nt-1 ----
            for nt in range(2, N_TILES):
                ps_prev = ps_l
                w8 = w8res.tile([P, N_DR, O_SUB, 2, P], fp8, tag="w8")
                wbf = wbres.tile([P, N_BF_CH, N_TILE], bf16, tag="wbf")
                for k in range(K_CH):
                    load_w_chunk(nt, k)
                if nt == 2:
                    for g in range(N_GROUPS):
                        evict(1, g, ps_prev[g], nc.gpsimd, burst=True)
                ps_l = []
                for g in range(N_GROUPS):
                    o_sub, th = divmod(g, T_HALF)
                    ps = psum.tile([P, N_TILE], f32, name=f"ps{g}", tag=f"ps{g}")
                    for k8 in range(N_DR):
                        nc.tensor.matmul(
                            ps[:],
                            w8[:, k8, o_sub, 0:2, :],
                            x8[:, k8, 0:2, th * N_TILE : (th + 1) * N_TILE],
                            start=(k8 == 0),
                            stop=False,
                            perf_mode=DR,
                        )
                    for kb in range(N_BF_CH):
                        nc.tensor.matmul(
                            ps[:],
                            wbf[:, kb, o_sub * P : (o_sub + 1) * P],
                            xbf[:, kb, th * N_TILE : (th + 1) * N_TILE],
                            start=False,
                            stop=(kb == N_BF_CH - 1),
                        )
                    evict(nt, g, ps, nc.sync)

    nc.compile()
    return nc


_FP8_NP = np.dtype(mybir.dt.np(mybir.dt.float8e4))
_BF16_NP = np.dtype(mybir.dt.np(mybir.dt.bfloat16))


def make_in_maps(x, weight, bias):
    x = np.asarray(x, dtype=np.float32)
    weight = np.asarray(weight, dtype=np.float32)
    bias = np.asarray(bias, dtype=np.float32)

    wT = np.ascontiguousarray(weight.T).astype(_BF16_NP)
    bias_col = np.ascontiguousarray(bias.reshape(D_OUT // P, P).T)
    in_maps = []
    for c in range(N_CORES):
        x2 = 2.0 * x[c * T_SHARD : (c + 1) * T_SHARD, :].T  # [D_IN, T]
        x8c = np.ascontiguousarray(
            x2[: N_FP8_CH * P]
            .reshape(N_DR, 2, P, T_SHARD)
            .transpose(2, 0, 1, 3)
        ).astype(_FP8_NP)
        xbfc = np.ascontiguousarray(
            x2[N_FP8_CH * P :]
            .reshape(N_BF_CH, P, T_SHARD)
            .transpose(1, 0, 2)
        ).astype(_BF16_NP)
        in_maps.append(
            {"x8in": x8c, "xbfin": xbfc, "wT": wT, "bias_col": bias_col}
        )
    return in_maps


def _spot_check(out, x, weight, bias):
    # transient device glitches were observed (~1/10 runs returns garbage);
    # verify a few rows against the host and signal a retry if corrupted
    rows = [1, T_FULL // 3, (2 * T_FULL) // 3, T_FULL - 2]
    ref = x[rows].astype(np.float64) @ np.sign(weight).astype(np.float64).T + bias
    err = np.linalg.norm(out[rows].astype(np.float64) - ref) / np.linalg.norm(ref)
    return err < 5e-2


def kernel(x, weight, bias):
    global _compiled
    if _compiled is None:
        _compiled = _build()
    nc = _compiled

    in_maps = make_in_maps(x, weight, bias)
    for _attempt in range(3):
        res = run_bass_kernel_spmd(nc, in_maps, core_ids=list(range(N_CORES)))
        out = np.concatenate(
            [np.ascontiguousarray(res.results[c]["outT"].T) for c in range(N_CORES)],
            axis=0,
        )
        if _spot_check(out, x, weight, bias):
            break
    return out

